# revision 6
# baseline (speedup 1.0000x reference)
"""GAT layer kernel for Trainium2 (8 NeuronCores, SPMD) — bf16 pipeline.

Math note: the per-destination softmax weights are only used through their
mean over each destination's incoming edges, and a softmax sums to 1, so
attn_w[i] = 1/deg[i] (0 if deg==0) exactly.  The output reduces to:

    out[i] = (agg[i] @ Wv.T + deg[i]*bv) * recip[i],  agg[i] = sum x[row[e]]

Device strategy (dst-node sharded, 49 windows of 128 dst nodes per core):
  - host sorts edges by (dst window, src half) and packs each window's
    edge list into T = T_LO + T_HI chunks of 128 slots; x is bf16.
  - per group of G windows: FOUR dma_gather calls (int16 indices, x
    split into two <32768-row halves) fetch x[row[e]] rows (256B each)
    into SBUF [128 slot, chunks*128] bf16.  The four gathers are spread
    over SWDGE queues 1,2,3,0: descriptor generation (~8ns/row) runs on
    a queue's own Q7 core pair, so queues 1-3 retire instantly on the
    Pool engine and generate concurrently; only queue 0 blocks.  Each
    DMASW sem lane is post-compile pinned to a single queue (ucode
    requirement).  Deep tile buffering (bufs=6) keeps the four
    generators saturated; this 4-way descriptor generation is the
    critical path (~24us per group).
  - per window one wide DVE op builds all T bf16 one-hots
    oh[p, t*128+j] = (j == col_local[p, t]); TensorE accumulates
    aggT[din, dst] += Xg_t^T @ oh_t into PSUM with bf16 matmuls.
  - epilogue (bf16 weights): out[dst, :] = (aggT^T @ WvT + deg^T x bv)
    * recip[dst], with the PSUM->SBUF copy and the recip scale on the
    Scalar engine.
"""

import os
import numpy as np

P = 128
NCORES = 8
N = 50000
XLO = 25088                   # rows in the low half of x (< 32768 for int16)
XHI = N - XLO
DIN = 128
DOUT = 128
WPC = 49                      # windows per core
NWIN = NCORES * WPC           # 392
NPAD = NWIN * P               # 50176
G = 5                         # windows per gather group

_last_exec_ns = None
_cache = {}


def _groups():
    out = []
    g0 = 0
    while g0 < WPC:
        out.append((g0, min(G, WPC - g0)))
        g0 += G
    return out


def _gsplits(Gg, T_LO, T_HI):
    """Per-group gather splits: (src_half, chunk_base, c0, c1, queue)."""
    nl = Gg * T_LO
    nh = Gg * T_HI
    la = (nl + 1) // 2
    ha = (nh + 1) // 2
    return [
        (0, 0, 0, la, 1),
        (0, 0, la, nl, 2),
        (1, nl, 0, ha, 3),
        (1, nl, ha, nh, 0),
    ]


def _ensure_ntff_hook():
    import sys
    import types
    if "antenv.axon_hooks" in sys.modules:
        return
    try:
        import antenv
        mod = types.ModuleType("antenv.axon_hooks")
        _h = [None]
        mod.set_axon_ntff_profile_hook = lambda hook: _h.__setitem__(0, hook)
        mod.get_axon_ntff_profile_hook = lambda: _h[0]
        sys.modules["antenv.axon_hooks"] = mod
        antenv.axon_hooks = mod
        from trn_agent_boot.trn_boot import _ntff_profile_via_ctypes
        hook = _ntff_profile_via_ctypes("/opt/axon/libaxon_pjrt.so")
        if hook is not None:
            mod.set_axon_ntff_profile_hook(hook)
    except Exception:
        pass


def _offsets(T):
    """Column offsets of the packed [P, CW] f32 constant tensor."""
    o = {}
    o["idx16"] = 0                        # int16 idx (wrapped), WPC*T*4 f32
    o["colb"] = o["idx16"] + WPC * T * 4  # col_local bf16, WPC*T/2 cols
    o["rec"] = o["colb"] + (WPC * T + 1) // 2
    o["wvtb"] = o["rec"] + WPC            # Wv.T bf16, DOUT/2 cols
    o["iotab"] = o["wvtb"] + DOUT // 2    # iota tiled bf16, T*P/2 cols
    o["bvb"] = o["iotab"] + T * P // 2    # bv bf16 at partition 0
    o["degb"] = o["bvb"] + DOUT // 2      # deg bf16 at partition 0
    o["CW"] = o["degb"] + WPC * P // 2
    return o


def _build(T, T_LO, T_HI):
    import concourse.bacc as bacc
    import concourse.mybir as mybir
    from concourse.tile import TileContext

    f32 = mybir.dt.float32
    bf16 = mybir.dt.bfloat16
    i16 = mybir.dt.int16

    o = _offsets(T)
    CW = o["CW"]

    nc = bacc.Bacc(None, target_bir_lowering=False, num_swdge_queues=4)
    xlo_d = nc.dram_tensor("xlo", [XLO, DIN], bf16, kind="ExternalInput")
    xhi_d = nc.dram_tensor("xhi", [XHI, DIN], bf16, kind="ExternalInput")
    NIDX = WPC * T * 4
    cidx_d = nc.dram_tensor("cidx", [P, NIDX], f32, kind="ExternalInput")
    crest_d = nc.dram_tensor("crest", [P, CW - NIDX], f32,
                             kind="ExternalInput")
    out_d = nc.dram_tensor("out", [WPC * P, DOUT], f32, kind="ExternalOutput")

    with TileContext(nc) as tc:
        with (
            tc.tile_pool(name="const", bufs=1) as cpool,
            tc.tile_pool(name="xg", bufs=6) as xgpool,
            tc.tile_pool(name="oh", bufs=6) as ohpool,
            tc.tile_pool(name="ep", bufs=2) as eppool,
            tc.tile_pool(name="ps", bufs=2, space="PSUM") as pspool,
            tc.tile_pool(name="po", bufs=2, space="PSUM") as popool,
        ):
            cidx_sb = cpool.tile([P, NIDX], f32, tag="cidx")
            crest_sb = cpool.tile([P, CW - NIDX], f32, tag="crest")
            # idx table first (the gathers only need this), rest loads in
            # the shadow of the first gather's descriptor generation; both
            # on HWDGE (sync) so the SWDGE lane round-robin stays aligned
            # with the periodic gather queue cycle below
            nc.sync.dma_start(out=cidx_sb[:], in_=cidx_d[:, :])
            nc.sync.dma_start(out=crest_sb[:], in_=crest_d[:, :])

            r = NIDX
            idx16_sb = cidx_sb[:].bitcast(i16)
            colb_sb = crest_sb[:, o["colb"] - r:o["rec"] - r].bitcast(bf16)
            rec_sb = crest_sb[:, o["rec"] - r:o["rec"] - r + WPC]
            wvtb_sb = crest_sb[:, o["wvtb"] - r:
                               o["wvtb"] - r + DOUT // 2].bitcast(bf16)
            iotab_sb = crest_sb[:, o["iotab"] - r:
                                o["iotab"] - r + T * P // 2].bitcast(bf16)
            bvb_sb = crest_sb[0:1, o["bvb"] - r:
                              o["bvb"] - r + DOUT // 2].bitcast(bf16)
            degb_sb = crest_sb[0:1, o["degb"] - r:
                               o["degb"] - r + WPC * P // 2].bitcast(bf16)

            goff16 = 0
            for g0, Gg in _groups():
                xg = xgpool.tile([P, Gg * T * P], bf16, tag="xg")
                xg3 = xg[:].rearrange("p (c e) -> p c e", e=P)
                # four gathers per group on SWDGE queues 1,2,3,0: queue 1-3
                # instructions retire immediately (desc-gen runs async on
                # their own Q7 core pairs); only queue 0 blocks the engine.
                # The fixed period-4 queue cycle keeps Tile's 8 DMASW sem
                # lanes queue-consistent (lane j%8 always sees queue
                # cycle[j%4]).
                for src_d, cbase, c0, c1, q in _gsplits(Gg, T_LO, T_HI):
                    ni = (c1 - c0) * P
                    nc.gpsimd.dma_gather(
                        out_ap=xg3[:, cbase + c0:cbase + c1, :],
                        in_ap=(xlo_d if src_d == 0 else xhi_d)[:, :],
                        idxs_ap=idx16_sb[:, goff16:goff16 + ni // 16],
                        num_idxs=ni,
                        num_idxs_reg=ni,
                        elem_size=DIN,
                        single_packet=False,
                        queue_num=q,
                    )
                    goff16 += ni // 16
                for wl in range(Gg):
                    w = g0 + wl
                    # all T one-hots for this window in one wide DVE op:
                    # oh[p, t, j] = (iota[j] == col_local[p, t])
                    oh = ohpool.tile([P, T * P], bf16, tag="oh")
                    nc.vector.tensor_tensor(
                        out=oh[:].rearrange("p (t j) -> p t j", j=P),
                        in0=iotab_sb[:].rearrange("p (t j) -> p t j", j=P),
                        in1=colb_sb[:, w * T:(w + 1) * T].to_broadcast(
                            [P, T, P]),
                        op=mybir.AluOpType.is_equal,
                    )
                    agg_ps = pspool.tile([P, P], f32, tag="agg")
                    for t in range(T):
                        if t < T_LO:
                            c = wl * T_LO + t
                        else:
                            c = Gg * T_LO + wl * T_HI + (t - T_LO)
                        nc.tensor.matmul(
                            out=agg_ps[:],
                            lhsT=xg[:, c * P:(c + 1) * P],
                            rhs=oh[:, t * P:(t + 1) * P],
                            start=(t == 0),
                            stop=(t == T - 1),
                        )
                    aggT_sb = eppool.tile([P, P], bf16, tag="aggT")
                    nc.scalar.copy(out=aggT_sb[:], in_=agg_ps[:])
                    out_ps = popool.tile([P, DOUT], f32, tag="outp")
                    nc.tensor.matmul(out=out_ps[:], lhsT=aggT_sb[:],
                                     rhs=wvtb_sb[:], start=True, stop=False)
                    nc.tensor.matmul(out=out_ps[:],
                                     lhsT=degb_sb[0:1, w * P:(w + 1) * P],
                                     rhs=bvb_sb[0:1, :], start=False,
                                     stop=True)
                    out_sb = eppool.tile([P, DOUT], f32, tag="outs")
                    nc.scalar.mul(out=out_sb[:], in_=out_ps[:],
                                  mul=rec_sb[:, w:w + 1])
                    nc.sync.dma_start(out=out_d[w * P:(w + 1) * P, :],
                                      in_=out_sb[:])
    nc.compile()
    # Rewrite each gather's SWDGE queue as a pure function of its ASSIGNED
    # DMASW sem lane, so every lane is incremented by exactly one queue
    # (the ucode tracks sem ownership per queue).  Queue 1-3 instructions
    # retire immediately on the Pool engine (desc-gen runs on their own Q7
    # core pairs); queue 0 blocks, so it gets 2 of the 8 lanes.
    lane_q = (1, 2, 3, 0)
    for bb in nc.m.functions[0].blocks:
        for inst in bb.instructions:
            if 'DMAGatherAnt' not in type(inst).__name__:
                continue
            lane = None
            si = inst.sync_info
            if si is not None:
                for u in si.on_update:
                    n = u.ant_name
                    if n and n.startswith('DMASW'):
                        lane = int(n[5:].split('_')[0])
            assert lane is not None, "gather without DMASW sem"
            inst.queue_num = lane_q[lane % 4]
    return nc


def _prep(row, col):
    """Host-side packing. Returns (T, T_LO, T_HI, per-core arrays)."""
    row = row.astype(np.int64)
    col = col.astype(np.int64)
    E = len(row)
    ishi = (row >= XLO).astype(np.int64)
    key = ((col >> 7) << 1) | ishi
    order = np.argsort(key, kind="stable")
    srow = row[order]
    scol = col[order]
    skey = key[order]

    deg = np.bincount(col, minlength=NPAD).astype(np.float32)
    recip = np.where(deg > 0, 1.0 / np.maximum(deg, 1.0), 0.0).astype(np.float32)

    cnt = np.bincount(key, minlength=2 * NWIN)
    lo_cnt, hi_cnt = cnt[0::2], cnt[1::2]
    T_LO = int(np.ceil(lo_cnt.max() / P))
    T_HI = int(np.ceil(hi_cnt.max() / P))
    T = T_LO + T_HI

    gstart = np.zeros(2 * NWIN + 1, np.int64)
    np.cumsum(cnt, out=gstart[1:])
    epos = np.arange(E, dtype=np.int64) - gstart[skey]
    p = epos % P
    tw = epos // P
    whalf = skey & 1
    win = skey >> 1
    tchunk = np.where(whalf == 1, tw + T_LO, tw)

    col_arr = np.full((NWIN, P, T), -1.0, np.float32)
    col_arr[win, p, tchunk] = (scol & (P - 1)).astype(np.float32)

    idx_lo = np.zeros((NWIN, T_LO * P), np.int16)
    idx_hi = np.zeros((NWIN, T_HI * P), np.int16)
    lo_m = whalf == 0
    hi_m = whalf == 1
    idx_lo[win[lo_m], epos[lo_m]] = srow[lo_m].astype(np.int16)
    idx_hi[win[hi_m], epos[hi_m]] = (srow[hi_m] - XLO).astype(np.int16)

    per_core = []
    for c in range(NCORES):
        wsl = slice(c * WPC, (c + 1) * WPC)
        # wrapped idx16 layout: per gather, index i at [i%16, i//16],
        # replicated across the 8 groups of 16 partitions; four gather
        # blocks per group matching _gsplits order
        cols16 = []
        for g0, Gg in _groups():
            wabs = c * WPC + g0
            halves = (idx_lo[wabs:wabs + Gg].reshape(-1),
                      idx_hi[wabs:wabs + Gg].reshape(-1))
            for src_d, cbase, c0, c1, q in _gsplits(Gg, T_LO, T_HI):
                flat = halves[src_d][c0 * P:c1 * P]
                wrapped = flat.reshape(-1, 16).T             # [16, ni/16]
                cols16.append(np.tile(wrapped, (8, 1)))      # [128, ni/16]
        idx16_map = np.concatenate(cols16, axis=1)           # [128, WPC*T*8]
        col_map = np.ascontiguousarray(
            col_arr[wsl].transpose(1, 0, 2).reshape(P, WPC * T))
        rec_map = np.ascontiguousarray(
            recip[c * WPC * P:(c + 1) * WPC * P].reshape(WPC, P).T)
        deg_map = np.ascontiguousarray(
            deg[c * WPC * P:(c + 1) * WPC * P].reshape(1, WPC * P))
        per_core.append((idx16_map, col_map, rec_map, deg_map))
    return T, T_LO, T_HI, per_core


def _put_bf16(arr, col_off, data_bf16):
    """Pack a bf16 [rows, n] block into f32 columns of arr at col_off."""
    rows, n = data_bf16.shape
    assert n % 2 == 0
    tmp = np.zeros((rows, n // 2), np.float32)
    tmp.view(np.uint16).reshape(rows, n)[:] = data_bf16.view(np.uint16)
    arr[:rows, col_off:col_off + n // 2] = tmp


def _pack_const(T, idx16_map, col_map, rec_map, deg_map, wvtb, bvb):
    """Returns (cidx, crest) arrays for the two constant tensors."""
    from ml_dtypes import bfloat16
    o = _offsets(T)
    r = WPC * T * 4
    assert idx16_map.shape == (P, WPC * T * 8)
    cidx = np.ascontiguousarray(idx16_map.view(np.float32))
    arr = np.zeros((P, o["CW"] - r), np.float32)
    _put_bf16(arr, o["colb"] - r, col_map.astype(bfloat16))
    arr[:, o["rec"] - r:o["rec"] - r + WPC] = rec_map
    _put_bf16(arr, o["wvtb"] - r, wvtb)
    iotab = np.broadcast_to(
        np.tile(np.arange(P, dtype=np.float32), T)[None, :],
        (P, T * P)).astype(bfloat16)
    _put_bf16(arr, o["iotab"] - r, np.ascontiguousarray(iotab))
    _put_bf16(arr, o["bvb"] - r, bvb)
    _put_bf16(arr, o["degb"] - r, deg_map.astype(bfloat16))
    return cidx, arr


def kernel(**inputs):
    global _last_exec_ns
    _ensure_ntff_hook()
    from concourse.bass_utils import run_bass_kernel_spmd
    from ml_dtypes import bfloat16

    x = np.ascontiguousarray(np.asarray(inputs["x"], dtype=np.float32))
    ei = np.asarray(inputs["edge_index"])
    row = np.asarray(ei[0]).astype(np.int64)
    col = np.asarray(ei[1]).astype(np.int64)
    Wv = np.asarray(inputs["Wv"], dtype=np.float32)
    bv = np.asarray(inputs["bv"], dtype=np.float32)

    xb = x.astype(bfloat16)
    wvtb = np.ascontiguousarray(Wv.T).astype(bfloat16)     # [DIN, DOUT]
    bvb = bv.reshape(1, DOUT).astype(bfloat16)

    T, T_LO, T_HI, per_core = _prep(row, col)

    key = (T, T_LO, T_HI)
    if key not in _cache:
        _cache[key] = _build(T, T_LO, T_HI)
    nc = _cache[key]

    xlo = np.ascontiguousarray(xb[:XLO])
    xhi = np.ascontiguousarray(xb[XLO:])
    in_maps = []
    for c in range(NCORES):
        cidx, crest = _pack_const(T, *per_core[c], wvtb, bvb)
        in_maps.append({"xlo": xlo, "xhi": xhi, "cidx": cidx,
                        "crest": crest})

    trace = bool(os.environ.get("GAT_TRACE"))
    res = run_bass_kernel_spmd(nc, in_maps, list(range(NCORES)), trace=trace)
    _last_exec_ns = res.exec_time_ns
    globals()["_last_res"] = res

    out = np.concatenate([res.results[c]["out"] for c in range(NCORES)], axis=0)
    return np.ascontiguousarray(out[:N])


# revision 7
# speedup vs baseline: 1.0131x; 1.0131x over previous
"""GAT layer kernel for Trainium2 (8 NeuronCores, SPMD) — bf16 pipeline.

Math note: the per-destination softmax weights are only used through their
mean over each destination's incoming edges, and a softmax sums to 1, so
attn_w[i] = 1/deg[i] (0 if deg==0) exactly.  The output reduces to:

    out[i] = (agg[i] @ Wv.T + deg[i]*bv) * recip[i],  agg[i] = sum x[row[e]]

Device strategy (dst-node sharded, 49 windows of 128 dst nodes per core):
  - host sorts edges by (dst window, src half) and packs each window's
    edge list into T = T_LO + T_HI chunks of 128 slots; x is bf16.
  - per group of G windows: FOUR dma_gather calls (int16 indices, x
    split into two <32768-row halves) fetch x[row[e]] rows (256B each)
    into SBUF [128 slot, chunks*128] bf16.  The four gathers are spread
    over SWDGE queues 1,2,3,0: descriptor generation (~8ns/row) runs on
    a queue's own Q7 core pair, so queues 1-3 retire instantly on the
    Pool engine and generate concurrently; only queue 0 blocks.  Each
    DMASW sem lane is post-compile pinned to a single queue (ucode
    requirement).  Deep tile buffering (bufs=6) keeps the four
    generators saturated; this 4-way descriptor generation is the
    critical path (~24us per group).
  - per window one wide DVE op builds all T bf16 one-hots
    oh[p, t*128+j] = (j == col_local[p, t]); TensorE accumulates
    aggT[din, dst] += Xg_t^T @ oh_t into PSUM with bf16 matmuls.
  - epilogue (bf16 weights): out[dst, :] = (aggT^T @ WvT + deg^T x bv)
    * recip[dst], with the PSUM->SBUF copy and the recip scale on the
    Scalar engine.
"""

import os
import numpy as np

P = 128
NCORES = 8
N = 50000
XLO = 25088                   # rows in the low half of x (< 32768 for int16)
XHI = N - XLO
DIN = 128
DOUT = 128
WPC = 49                      # windows per core
NWIN = NCORES * WPC           # 392
NPAD = NWIN * P               # 50176
G = 5                         # windows per gather group

_last_exec_ns = None
_cache = {}


def _groups():
    # tapered tail: the final 1-window group drains the pipeline in ~6us
    # (gather gen + DMA + compute) instead of a full group's ~24us
    sizes = [G] * 9 + [3, 1]
    assert sum(sizes) == WPC
    out = []
    g0 = 0
    for s in sizes:
        out.append((g0, s))
        g0 += s
    return out


def _gsplits(Gg, T_LO, T_HI):
    """Per-group gather splits: (src_half, chunk_base, c0, c1, queue)."""
    nl = Gg * T_LO
    nh = Gg * T_HI
    la = (nl + 1) // 2
    ha = (nh + 1) // 2
    return [
        (0, 0, 0, la, 1),
        (0, 0, la, nl, 2),
        (1, nl, 0, ha, 3),
        (1, nl, ha, nh, 0),
    ]


def _ensure_ntff_hook():
    import sys
    import types
    if "antenv.axon_hooks" in sys.modules:
        return
    try:
        import antenv
        mod = types.ModuleType("antenv.axon_hooks")
        _h = [None]
        mod.set_axon_ntff_profile_hook = lambda hook: _h.__setitem__(0, hook)
        mod.get_axon_ntff_profile_hook = lambda: _h[0]
        sys.modules["antenv.axon_hooks"] = mod
        antenv.axon_hooks = mod
        from trn_agent_boot.trn_boot import _ntff_profile_via_ctypes
        hook = _ntff_profile_via_ctypes("/opt/axon/libaxon_pjrt.so")
        if hook is not None:
            mod.set_axon_ntff_profile_hook(hook)
    except Exception:
        pass


def _offsets(T):
    """Column offsets of the packed [P, CW] f32 constant tensor."""
    o = {}
    o["idx16"] = 0                        # int16 idx (wrapped), WPC*T*4 f32
    o["colb"] = o["idx16"] + WPC * T * 4  # col_local bf16, WPC*T/2 cols
    o["rec"] = o["colb"] + (WPC * T + 1) // 2
    o["wvtb"] = o["rec"] + WPC            # Wv.T bf16, DOUT/2 cols
    o["iotab"] = o["wvtb"] + DOUT // 2    # iota tiled bf16, T*P/2 cols
    o["bvb"] = o["iotab"] + T * P // 2    # bv bf16 at partition 0
    o["degb"] = o["bvb"] + DOUT // 2      # deg bf16 at partition 0
    o["CW"] = o["degb"] + WPC * P // 2
    return o


def _build(T, T_LO, T_HI):
    import concourse.bacc as bacc
    import concourse.mybir as mybir
    from concourse.tile import TileContext

    f32 = mybir.dt.float32
    bf16 = mybir.dt.bfloat16
    i16 = mybir.dt.int16

    o = _offsets(T)
    CW = o["CW"]

    nc = bacc.Bacc(None, target_bir_lowering=False, num_swdge_queues=4)
    xlo_d = nc.dram_tensor("xlo", [XLO, DIN], bf16, kind="ExternalInput")
    xhi_d = nc.dram_tensor("xhi", [XHI, DIN], bf16, kind="ExternalInput")
    NIDX = WPC * T * 4
    cidx_d = nc.dram_tensor("cidx", [P, NIDX], f32, kind="ExternalInput")
    crest_d = nc.dram_tensor("crest", [P, CW - NIDX], f32,
                             kind="ExternalInput")
    out_d = nc.dram_tensor("out", [WPC * P, DOUT], f32, kind="ExternalOutput")

    with TileContext(nc) as tc:
        with (
            tc.tile_pool(name="const", bufs=1) as cpool,
            tc.tile_pool(name="xg", bufs=6) as xgpool,
            tc.tile_pool(name="oh", bufs=6) as ohpool,
            tc.tile_pool(name="ep", bufs=2) as eppool,
            tc.tile_pool(name="ps", bufs=2, space="PSUM") as pspool,
            tc.tile_pool(name="po", bufs=2, space="PSUM") as popool,
        ):
            cidx_sb = cpool.tile([P, NIDX], f32, tag="cidx")
            crest_sb = cpool.tile([P, CW - NIDX], f32, tag="crest")
            # idx table first (the gathers only need this), rest loads in
            # the shadow of the first gather's descriptor generation; both
            # on HWDGE (sync) so the SWDGE lane round-robin stays aligned
            # with the periodic gather queue cycle below
            nc.sync.dma_start(out=cidx_sb[:], in_=cidx_d[:, :])
            nc.sync.dma_start(out=crest_sb[:], in_=crest_d[:, :])

            r = NIDX
            idx16_sb = cidx_sb[:].bitcast(i16)
            colb_sb = crest_sb[:, o["colb"] - r:o["rec"] - r].bitcast(bf16)
            rec_sb = crest_sb[:, o["rec"] - r:o["rec"] - r + WPC]
            wvtb_sb = crest_sb[:, o["wvtb"] - r:
                               o["wvtb"] - r + DOUT // 2].bitcast(bf16)
            iotab_sb = crest_sb[:, o["iotab"] - r:
                                o["iotab"] - r + T * P // 2].bitcast(bf16)
            bvb_sb = crest_sb[0:1, o["bvb"] - r:
                              o["bvb"] - r + DOUT // 2].bitcast(bf16)
            degb_sb = crest_sb[0:1, o["degb"] - r:
                               o["degb"] - r + WPC * P // 2].bitcast(bf16)

            goff16 = 0
            for g0, Gg in _groups():
                xg = xgpool.tile([P, Gg * T * P], bf16, tag="xg")
                xg3 = xg[:].rearrange("p (c e) -> p c e", e=P)
                # four gathers per group on SWDGE queues 1,2,3,0: queue 1-3
                # instructions retire immediately (desc-gen runs async on
                # their own Q7 core pairs); only queue 0 blocks the engine.
                # The fixed period-4 queue cycle keeps Tile's 8 DMASW sem
                # lanes queue-consistent (lane j%8 always sees queue
                # cycle[j%4]).
                for src_d, cbase, c0, c1, q in _gsplits(Gg, T_LO, T_HI):
                    ni = (c1 - c0) * P
                    nc.gpsimd.dma_gather(
                        out_ap=xg3[:, cbase + c0:cbase + c1, :],
                        in_ap=(xlo_d if src_d == 0 else xhi_d)[:, :],
                        idxs_ap=idx16_sb[:, goff16:goff16 + ni // 16],
                        num_idxs=ni,
                        num_idxs_reg=ni,
                        elem_size=DIN,
                        single_packet=False,
                        queue_num=q,
                    )
                    goff16 += ni // 16
                for wl in range(Gg):
                    w = g0 + wl
                    # all T one-hots for this window in one wide DVE op:
                    # oh[p, t, j] = (iota[j] == col_local[p, t])
                    oh = ohpool.tile([P, T * P], bf16, tag="oh")
                    nc.vector.tensor_tensor(
                        out=oh[:].rearrange("p (t j) -> p t j", j=P),
                        in0=iotab_sb[:].rearrange("p (t j) -> p t j", j=P),
                        in1=colb_sb[:, w * T:(w + 1) * T].to_broadcast(
                            [P, T, P]),
                        op=mybir.AluOpType.is_equal,
                    )
                    agg_ps = pspool.tile([P, P], f32, tag="agg")
                    for t in range(T):
                        if t < T_LO:
                            c = wl * T_LO + t
                        else:
                            c = Gg * T_LO + wl * T_HI + (t - T_LO)
                        nc.tensor.matmul(
                            out=agg_ps[:],
                            lhsT=xg[:, c * P:(c + 1) * P],
                            rhs=oh[:, t * P:(t + 1) * P],
                            start=(t == 0),
                            stop=(t == T - 1),
                        )
                    aggT_sb = eppool.tile([P, P], bf16, tag="aggT")
                    nc.scalar.copy(out=aggT_sb[:], in_=agg_ps[:])
                    out_ps = popool.tile([P, DOUT], f32, tag="outp")
                    nc.tensor.matmul(out=out_ps[:], lhsT=aggT_sb[:],
                                     rhs=wvtb_sb[:], start=True, stop=False)
                    nc.tensor.matmul(out=out_ps[:],
                                     lhsT=degb_sb[0:1, w * P:(w + 1) * P],
                                     rhs=bvb_sb[0:1, :], start=False,
                                     stop=True)
                    out_sb = eppool.tile([P, DOUT], f32, tag="outs")
                    nc.scalar.mul(out=out_sb[:], in_=out_ps[:],
                                  mul=rec_sb[:, w:w + 1])
                    nc.sync.dma_start(out=out_d[w * P:(w + 1) * P, :],
                                      in_=out_sb[:])
    nc.compile()
    # Rewrite each gather's SWDGE queue as a pure function of its ASSIGNED
    # DMASW sem lane, so every lane is incremented by exactly one queue
    # (the ucode tracks sem ownership per queue).  Queue 1-3 instructions
    # retire immediately on the Pool engine (desc-gen runs on their own Q7
    # core pairs); queue 0 blocks, so it gets 2 of the 8 lanes.
    lane_q = (1, 2, 3, 0)
    for bb in nc.m.functions[0].blocks:
        for inst in bb.instructions:
            if 'DMAGatherAnt' not in type(inst).__name__:
                continue
            lane = None
            si = inst.sync_info
            if si is not None:
                for u in si.on_update:
                    n = u.ant_name
                    if n and n.startswith('DMASW'):
                        lane = int(n[5:].split('_')[0])
            assert lane is not None, "gather without DMASW sem"
            inst.queue_num = lane_q[lane % 4]
    return nc


def _prep(row, col):
    """Host-side packing. Returns (T, T_LO, T_HI, per-core arrays)."""
    row = row.astype(np.int64)
    col = col.astype(np.int64)
    E = len(row)
    ishi = (row >= XLO).astype(np.int64)
    key = ((col >> 7) << 1) | ishi
    order = np.argsort(key, kind="stable")
    srow = row[order]
    scol = col[order]
    skey = key[order]

    deg = np.bincount(col, minlength=NPAD).astype(np.float32)
    recip = np.where(deg > 0, 1.0 / np.maximum(deg, 1.0), 0.0).astype(np.float32)

    cnt = np.bincount(key, minlength=2 * NWIN)
    lo_cnt, hi_cnt = cnt[0::2], cnt[1::2]
    T_LO = int(np.ceil(lo_cnt.max() / P))
    T_HI = int(np.ceil(hi_cnt.max() / P))
    T = T_LO + T_HI

    gstart = np.zeros(2 * NWIN + 1, np.int64)
    np.cumsum(cnt, out=gstart[1:])
    epos = np.arange(E, dtype=np.int64) - gstart[skey]
    p = epos % P
    tw = epos // P
    whalf = skey & 1
    win = skey >> 1
    tchunk = np.where(whalf == 1, tw + T_LO, tw)

    col_arr = np.full((NWIN, P, T), -1.0, np.float32)
    col_arr[win, p, tchunk] = (scol & (P - 1)).astype(np.float32)

    idx_lo = np.zeros((NWIN, T_LO * P), np.int16)
    idx_hi = np.zeros((NWIN, T_HI * P), np.int16)
    lo_m = whalf == 0
    hi_m = whalf == 1
    idx_lo[win[lo_m], epos[lo_m]] = srow[lo_m].astype(np.int16)
    idx_hi[win[hi_m], epos[hi_m]] = (srow[hi_m] - XLO).astype(np.int16)

    per_core = []
    for c in range(NCORES):
        wsl = slice(c * WPC, (c + 1) * WPC)
        # wrapped idx16 layout: per gather, index i at [i%16, i//16],
        # replicated across the 8 groups of 16 partitions; four gather
        # blocks per group matching _gsplits order
        cols16 = []
        for g0, Gg in _groups():
            wabs = c * WPC + g0
            halves = (idx_lo[wabs:wabs + Gg].reshape(-1),
                      idx_hi[wabs:wabs + Gg].reshape(-1))
            for src_d, cbase, c0, c1, q in _gsplits(Gg, T_LO, T_HI):
                flat = halves[src_d][c0 * P:c1 * P]
                wrapped = flat.reshape(-1, 16).T             # [16, ni/16]
                cols16.append(np.tile(wrapped, (8, 1)))      # [128, ni/16]
        idx16_map = np.concatenate(cols16, axis=1)           # [128, WPC*T*8]
        col_map = np.ascontiguousarray(
            col_arr[wsl].transpose(1, 0, 2).reshape(P, WPC * T))
        rec_map = np.ascontiguousarray(
            recip[c * WPC * P:(c + 1) * WPC * P].reshape(WPC, P).T)
        deg_map = np.ascontiguousarray(
            deg[c * WPC * P:(c + 1) * WPC * P].reshape(1, WPC * P))
        per_core.append((idx16_map, col_map, rec_map, deg_map))
    return T, T_LO, T_HI, per_core


def _put_bf16(arr, col_off, data_bf16):
    """Pack a bf16 [rows, n] block into f32 columns of arr at col_off."""
    rows, n = data_bf16.shape
    assert n % 2 == 0
    tmp = np.zeros((rows, n // 2), np.float32)
    tmp.view(np.uint16).reshape(rows, n)[:] = data_bf16.view(np.uint16)
    arr[:rows, col_off:col_off + n // 2] = tmp


def _pack_const(T, idx16_map, col_map, rec_map, deg_map, wvtb, bvb):
    """Returns (cidx, crest) arrays for the two constant tensors."""
    from ml_dtypes import bfloat16
    o = _offsets(T)
    r = WPC * T * 4
    assert idx16_map.shape == (P, WPC * T * 8)
    cidx = np.ascontiguousarray(idx16_map.view(np.float32))
    arr = np.zeros((P, o["CW"] - r), np.float32)
    _put_bf16(arr, o["colb"] - r, col_map.astype(bfloat16))
    arr[:, o["rec"] - r:o["rec"] - r + WPC] = rec_map
    _put_bf16(arr, o["wvtb"] - r, wvtb)
    iotab = np.broadcast_to(
        np.tile(np.arange(P, dtype=np.float32), T)[None, :],
        (P, T * P)).astype(bfloat16)
    _put_bf16(arr, o["iotab"] - r, np.ascontiguousarray(iotab))
    _put_bf16(arr, o["bvb"] - r, bvb)
    _put_bf16(arr, o["degb"] - r, deg_map.astype(bfloat16))
    return cidx, arr


def kernel(**inputs):
    global _last_exec_ns
    _ensure_ntff_hook()
    from concourse.bass_utils import run_bass_kernel_spmd
    from ml_dtypes import bfloat16

    x = np.ascontiguousarray(np.asarray(inputs["x"], dtype=np.float32))
    ei = np.asarray(inputs["edge_index"])
    row = np.asarray(ei[0]).astype(np.int64)
    col = np.asarray(ei[1]).astype(np.int64)
    Wv = np.asarray(inputs["Wv"], dtype=np.float32)
    bv = np.asarray(inputs["bv"], dtype=np.float32)

    xb = x.astype(bfloat16)
    wvtb = np.ascontiguousarray(Wv.T).astype(bfloat16)     # [DIN, DOUT]
    bvb = bv.reshape(1, DOUT).astype(bfloat16)

    T, T_LO, T_HI, per_core = _prep(row, col)

    key = (T, T_LO, T_HI)
    if key not in _cache:
        _cache[key] = _build(T, T_LO, T_HI)
    nc = _cache[key]

    xlo = np.ascontiguousarray(xb[:XLO])
    xhi = np.ascontiguousarray(xb[XLO:])
    in_maps = []
    for c in range(NCORES):
        cidx, crest = _pack_const(T, *per_core[c], wvtb, bvb)
        in_maps.append({"xlo": xlo, "xhi": xhi, "cidx": cidx,
                        "crest": crest})

    trace = bool(os.environ.get("GAT_TRACE"))
    res = run_bass_kernel_spmd(nc, in_maps, list(range(NCORES)), trace=trace)
    _last_exec_ns = res.exec_time_ns
    globals()["_last_res"] = res

    out = np.concatenate([res.results[c]["out"] for c in range(NCORES)], axis=0)
    return np.ascontiguousarray(out[:N])
